# revision 5
# baseline (speedup 1.0000x reference)
"""Trainium2 Bass kernel for nn_ContrastiveLoss (sparse cross-attention t2i loss).

Strategy (sharding_hint): shard the caption (txt) batch axis across the 8
cores — 8 captions per core, processed in 4 pairs of 2 (two 50-word captions
packed into the 128-partition dim at offsets 0 / 64). The image batch
(64 imgs x 36 regions = 2304 "bp" columns) is replicated.

Math reformulation (avoids materializing the (Bt,Bi,P,D) weighted context):
  raw[l,bp]  = s_t[l] . im[bp]                    (PE, D=1024 contraction)
  leak       = max(raw, 0.1*raw)                  (LeakyReLU 0.1)
  xhat       = leak / (||leak||_2 over p + eps)   (region-axis l2norm)
  E          = exp(9*xhat + biasmask)             (word mask via -40 bias)
  denom[bp]  = sum_l E  (ones-matmul on PE)       (softmax denominator)
  G          = E * (E > denom/cap_len)            (threshold sparsify,
                                                   unnormalized attention)
  w12[bp]    = sum_l G*raw                        (= denom * <im, wc>)
  w2sq[bp]   = ||C^T G||^2, K=s s^T=C C^T (chol)  (= denom^2 * ||wc||^2)
  cos        = w12 / max(w1 * sqrt(w2sq), eps)    (denom scale cancels)
The diagonal-margin loss over the gathered (64,64) score matrix is computed
on the host (a few thousand flops).
"""

import numpy as np
from contextlib import ExitStack

import concourse.bass as bass
import concourse.bacc as bacc
import concourse.tile as tile
from concourse import mybir, bass_utils

F32 = mybir.dt.float32
AF = mybir.ActivationFunctionType
OP = mybir.AluOpType

B = 64          # batch (captions == images)
P = 36          # regions per image
D = 1024        # feature dim
L = 50          # padded words per caption
BP = B * P      # 2304 score columns
NCORES = 8
TLOC = B // NCORES   # 8 captions per core
NPAIR = TLOC // 2    # 4 caption pairs per core
KCH = D // 128       # 8 contraction chunks
LAM = 9.0
EPS = 1e-8
MARGIN = 0.2
MASK_BIAS = -40.0

# N-tiling of the 2304-wide free axis (PSUM bank = 512 fp32)
NSL = [(0, 512), (512, 512), (1024, 512), (1536, 512), (2048, 256)]

# matmul operand dtype for the big step-1 matmul and the small reductions.
# float32r = full-rate (1 cyc/col) reduced-precision fp32; float32 = 4 cyc/col.
MM_BIG = F32
MM_RED = F32


def _mm(ap, dt_):
    return ap.bitcast(dt_) if dt_ != F32 else ap


def _build_device_program():
    nc = bacc.Bacc("TRN2", target_bir_lowering=False, debug=False)

    imT = nc.dram_tensor("imT", [D, BP], F32, kind="ExternalInput")
    sT = nc.dram_tensor("sT", [NPAIR, D, 128], F32, kind="ExternalInput")
    cch = nc.dram_tensor("cch", [NPAIR, 128, 128], F32, kind="ExternalInput")
    biasM = nc.dram_tensor("biasM", [128, NPAIR], F32, kind="ExternalInput")
    invl = nc.dram_tensor("invl", [2, NPAIR], F32, kind="ExternalInput")
    onesM2d = nc.dram_tensor("onesM2", [128, 2], F32, kind="ExternalInput")
    bcast2d = nc.dram_tensor("bcast2", [2, 128], F32, kind="ExternalInput")
    w12o = nc.dram_tensor("w12o", [TLOC, BP], F32, kind="ExternalOutput")
    w2o = nc.dram_tensor("w2o", [TLOC, BP], F32, kind="ExternalOutput")

    with tile.TileContext(nc) as tc:
        with ExitStack() as ctx:
            _body(ctx, tc, imT, sT, cch, biasM, invl, onesM2d, bcast2d, w12o, w2o)
    nc.compile()
    return nc


def _body(ctx, tc, imT, sT, cch, biasM, invl, onesM2d, bcast2d, w12o, w2o):
    nc = tc.nc
    consts = ctx.enter_context(tc.tile_pool(name="consts", bufs=1))
    imtp = ctx.enter_context(tc.tile_pool(name="imtp", bufs=KCH))
    stp = ctx.enter_context(tc.tile_pool(name="stp", bufs=2))
    cchp = ctx.enter_context(tc.tile_pool(name="cchp", bufs=2))
    big = ctx.enter_context(tc.tile_pool(name="big", bufs=2))
    scr = ctx.enter_context(tc.tile_pool(name="scr", bufs=2))
    small = ctx.enter_context(tc.tile_pool(name="small", bufs=3))
    rows = ctx.enter_context(tc.tile_pool(name="rows", bufs=2))
    rawp = ctx.enter_context(tc.tile_pool(name="rawp", bufs=2, space="PSUM"))
    colp = ctx.enter_context(tc.tile_pool(name="colp", bufs=2, space="PSUM"))
    rowp = ctx.enter_context(tc.tile_pool(name="rowp", bufs=2, space="PSUM"))

    # --- constants ---
    bias_sb = consts.tile([128, NPAIR], F32)
    nc.sync.dma_start(bias_sb, biasM.ap())
    invl_sb = consts.tile([2, NPAIR], F32)
    nc.sync.dma_start(invl_sb, invl.ap())

    # ones over caption-row blocks: lhsT for the partition-axis reductions
    onesM2 = consts.tile([128, 2], F32)
    nc.sync.dma_start(onesM2, onesM2d.ap())
    # 2->128 partition broadcast lhsT (row r of T to partition half r)
    bcast2 = consts.tile([2, 128], F32)
    nc.sync.dma_start(bcast2, bcast2d.ap())

    # --- image features, pre-transposed to [D, BP], as 8 K-chunks ---
    imt = []
    imT_r = imT.ap().rearrange("(k p) n -> k p n", p=128)
    for k in range(KCH):
        t = imtp.tile([128, BP], F32, name=f"imt{k}", tag="imt")
        nc.sync.dma_start(t, imT_r[k])
        imt.append(t)

    for pr in range(NPAIR):
        # caption-pair inputs: s^T padded into a [D, 128] block layout
        sT_sb = stp.tile([128, KCH, 128], F32, name="sT_sb")
        nc.sync.dma_start(sT_sb, sT.ap()[pr].rearrange("(k q) m -> q k m", q=128))
        cch_sb = cchp.tile([128, 128], F32, name="cch_sb")
        nc.sync.dma_start(cch_sb, cch.ap()[pr])

        # --- step 1: raw[l, bp] = s_t . im  (both captions at once) ---
        raw_sb = big.tile([128, BP], F32, name="raw_sb")
        for (n0, nn) in NSL:
            ps = rawp.tile([128, 512], F32, name="rawps", tag="rawps")[:, :nn]
            for k in range(KCH):
                nc.tensor.matmul(
                    ps,
                    lhsT=_mm(sT_sb[:, k, :], MM_BIG),
                    rhs=_mm(imt[k][:, n0:n0 + nn], MM_BIG),
                    start=(k == 0),
                    stop=(k == KCH - 1),
                )
            nc.scalar.copy(raw_sb[:, n0:n0 + nn], ps)

        # --- leaky relu: leak = max(0.1*raw, raw) (one fused op) ---
        leak = big.tile([128, BP], F32, name="leak")
        nc.vector.scalar_tensor_tensor(
            leak, raw_sb, 0.1, raw_sb, op0=OP.mult, op1=OP.max
        )

        # --- l2 norm over the 36-region groups ---
        xsq = scr.tile([128, BP], F32, name="xsq", tag="scratch")
        nc.scalar.activation(xsq, leak, AF.Square)
        nsum = small.tile([128, B], F32, name="nsum")
        nc.vector.reduce_sum(
            nsum,
            xsq.rearrange("p (b q) -> p b q", q=P),
            axis=mybir.AxisListType.X,
        )
        rt = small.tile([128, B], F32, name="rt")
        nc.scalar.activation(rt, nsum, AF.Sqrt)
        nc.vector.tensor_scalar_add(rt, rt, EPS)
        fac = small.tile([128, B], F32, name="fac")
        nc.vector.reciprocal(fac, rt)
        nc.vector.tensor_scalar_mul(fac, fac, LAM)
        # xhat*lam, in place over leak
        leak3 = leak.rearrange("p (b q) -> p b q", q=P)
        nc.vector.tensor_mul(leak3, leak3, fac[:, :, None].to_broadcast([128, B, P]))

        # --- E = exp(lam*xhat + mask_bias) ---
        E = big.tile([128, BP], F32, name="E")
        nc.scalar.activation(E, leak, AF.Exp, bias=bias_sb[:, pr:pr + 1], scale=1.0)

        # --- softmax denominator rows (per caption) via ones-matmul ---
        denom = rows.tile([2, BP], F32, name="denom", tag="rows")
        for (n0, nn) in NSL:
            rp = rowp.tile([2, 512], F32, name="rowps", tag="rowps")[:, :nn]
            nc.tensor.matmul(
                rp, lhsT=_mm(onesM2, MM_RED), rhs=_mm(E[:, n0:n0 + nn], MM_RED)
            )
            nc.vector.tensor_copy(denom[:, n0:n0 + nn], rp)
        # threshold rows T = denom / cap_len (in place)
        nc.vector.tensor_scalar_mul(denom, denom, invl_sb[:, pr:pr + 1])

        # --- threshold: G = E * (E > T_broadcast) ---
        Gm = scr.tile([128, BP], F32, name="Gm", tag="scratch")
        for (n0, nn) in NSL:
            tb = colp.tile([128, 512], F32, name="colps", tag="colps")[:, :nn]
            nc.tensor.matmul(
                tb, lhsT=_mm(bcast2, MM_RED), rhs=_mm(denom[:, n0:n0 + nn], MM_RED)
            )
            nc.vector.tensor_tensor(
                Gm[:, n0:n0 + nn], E[:, n0:n0 + nn], tb, op=OP.is_gt
            )
        G = big.tile([128, BP], F32, name="G")
        nc.vector.tensor_mul(G, E, Gm)

        # --- w12 = sum_l G * raw ---
        prod = big.tile([128, BP], F32, name="prod", tag="leak")
        nc.gpsimd.tensor_mul(prod, G, raw_sb)
        w12sb = rows.tile([2, BP], F32, name="w12sb", tag="rows")
        for (n0, nn) in NSL:
            rp = rowp.tile([2, 512], F32, name="rowps", tag="rowps")[:, :nn]
            nc.tensor.matmul(
                rp, lhsT=_mm(onesM2, MM_RED), rhs=_mm(prod[:, n0:n0 + nn], MM_RED)
            )
            nc.vector.tensor_copy(w12sb[:, n0:n0 + nn], rp)
        nc.sync.dma_start(w12o.ap()[2 * pr:2 * pr + 2, :], w12sb)

        # --- w2sq = sum_l (C^T G)^2 ---
        sqV = scr.tile([128, BP], F32, name="sqV", tag="scratch")
        for (n0, nn) in NSL:
            vp = colp.tile([128, 512], F32, name="colps", tag="colps")[:, :nn]
            nc.tensor.matmul(
                vp, lhsT=_mm(cch_sb, MM_BIG), rhs=_mm(G[:, n0:n0 + nn], MM_BIG)
            )
            nc.scalar.activation(sqV[:, n0:n0 + nn], vp, AF.Square)
        w2sb = rows.tile([2, BP], F32, name="w2sb", tag="rows")
        for (n0, nn) in NSL:
            rp = rowp.tile([2, 512], F32, name="rowps", tag="rowps")[:, :nn]
            nc.tensor.matmul(
                rp, lhsT=_mm(onesM2, MM_RED), rhs=_mm(sqV[:, n0:n0 + nn], MM_RED)
            )
            nc.vector.tensor_copy(w2sb[:, n0:n0 + nn], rp)
        nc.sync.dma_start(w2o.ap()[2 * pr:2 * pr + 2, :], w2sb)


_ONESM2 = np.zeros((128, 2), np.float32)
_ONESM2[0:L, 0] = 1.0
_ONESM2[64:64 + L, 1] = 1.0
_BCAST2 = np.zeros((2, 128), np.float32)
_BCAST2[0, 0:64] = 1.0
_BCAST2[1, 64:128] = 1.0

_CACHE = {}


def _get_program():
    if "nc" not in _CACHE:
        _CACHE["nc"] = _build_device_program()
    return _CACHE["nc"]


def _host_inputs(im, s, cl):
    """Build per-core in_maps (host-side sharding + layout prep)."""
    imT = np.ascontiguousarray(im.reshape(B * P, D).T)
    in_maps = []
    for c in range(NCORES):
        s_loc = s[c * TLOC:(c + 1) * TLOC]
        cl_loc = cl[c * TLOC:(c + 1) * TLOC]
        sT = np.zeros((NPAIR, D, 128), np.float32)
        cc = np.zeros((NPAIR, 128, 128), np.float32)
        bm = np.full((128, NPAIR), MASK_BIAS, np.float32)
        iv = np.zeros((2, NPAIR), np.float32)
        for pr in range(NPAIR):
            for j in (0, 1):
                t = 2 * pr + j
                off = 64 * j
                st = s_loc[t]
                sT[pr, :, off:off + L] = st.T
                K = (st.astype(np.float64) @ st.astype(np.float64).T)
                C = np.linalg.cholesky(K).astype(np.float32)
                cc[pr, off:off + L, off:off + L] = C
                bm[off:off + L, pr] = np.where(
                    np.arange(L) < cl_loc[t], 0.0, MASK_BIAS
                ).astype(np.float32)
                iv[j, pr] = np.float32(1.0 / float(cl_loc[t]))
        in_maps.append({"imT": imT, "sT": sT, "cch": cc, "biasM": bm, "invl": iv,
                        "onesM2": _ONESM2, "bcast2": _BCAST2})
    return in_maps


def _host_tail(im, w12, w2sq):
    imf = im.reshape(B * P, D)
    w1 = np.sqrt(np.sum(imf * imf, axis=1, dtype=np.float32))
    w2 = np.sqrt(np.maximum(w2sq, 0.0))
    cos = w12 / np.maximum(w1[None, :] * w2, np.float32(EPS))
    cosr = cos.reshape(B, B, P)
    scores = np.sort(cosr, axis=-1)[..., P // 3:].mean(axis=-1, dtype=np.float32)
    d = np.diag(scores).copy()
    cs = np.maximum(np.float32(MARGIN) + scores - d[:, None], 0.0)
    ci = np.maximum(np.float32(MARGIN) + scores - d[None, :], 0.0)
    np.fill_diagonal(cs, 0.0)
    np.fill_diagonal(ci, 0.0)
    loss = cs.max(axis=1).sum(dtype=np.float32) + ci.max(axis=0).sum(dtype=np.float32)
    return np.asarray(loss, dtype=np.float32)


def kernel(im, s, cap_lens, _profile=False):
    im = np.ascontiguousarray(np.asarray(im, dtype=np.float32))
    s = np.ascontiguousarray(np.asarray(s, dtype=np.float32))
    cl = np.asarray(cap_lens).astype(np.int64)

    nc = _get_program()
    in_maps = _host_inputs(im, s, cl)
    kw = dict(trace=True) if _profile else {}
    res = bass_utils.run_bass_kernel_spmd(
        nc, in_maps, core_ids=list(range(NCORES)), **kw
    )
    w12 = np.concatenate([res.results[c]["w12o"] for c in range(NCORES)], axis=0)
    w2sq = np.concatenate([res.results[c]["w2o"] for c in range(NCORES)], axis=0)
    out = _host_tail(im, w12, w2sq)
    if _profile:
        return out, res
    return out


# revision 7
# speedup vs baseline: 1.6119x; 1.6119x over previous
"""Trainium2 Bass kernel for nn_ContrastiveLoss (sparse cross-attention t2i loss).

Strategy (sharding_hint): shard the caption (txt) batch axis across the 8
cores — 8 captions per core, processed in 4 pairs of 2 (two 50-word captions
packed into the 128-partition dim at offsets 0 / 64). The image batch
(64 imgs x 36 regions = 2304 "bp" columns) is replicated.

Math reformulation (avoids materializing the (Bt,Bi,P,D) weighted context):
  raw[l,bp]  = s_t[l] . im[bp]                    (PE, D=1024 contraction)
  leak       = max(raw, 0.1*raw)                  (LeakyReLU 0.1)
  xhat       = leak / (||leak||_2 over p + eps)   (region-axis l2norm)
  E          = exp(9*xhat + biasmask)             (word mask via -40 bias)
  denom[bp]  = sum_l E  (ones-matmul on PE)       (softmax denominator)
  G          = E * (E > denom/cap_len)            (threshold sparsify,
                                                   unnormalized attention)
  w12[bp]    = sum_l G*raw                        (= denom * <im, wc>)
  w2sq[bp]   = ||C^T G||^2, K=s s^T=C C^T (chol)  (= denom^2 * ||wc||^2)
  cos        = w12 / max(w1 * sqrt(w2sq), eps)    (denom scale cancels)
The diagonal-margin loss over the gathered (64,64) score matrix is computed
on the host (a few thousand flops).
"""

import numpy as np
from contextlib import ExitStack

import concourse.bass as bass
import concourse.bacc as bacc
import concourse.tile as tile
from concourse import mybir, bass_utils

F32 = mybir.dt.float32
AF = mybir.ActivationFunctionType
OP = mybir.AluOpType

B = 64          # batch (captions == images)
P = 36          # regions per image
D = 1024        # feature dim
L = 50          # padded words per caption
BP = B * P      # 2304 score columns
NCORES = 8
TLOC = B // NCORES   # 8 captions per core
NPAIR = TLOC // 2    # 4 caption pairs per core
KCH = D // 128       # 8 contraction chunks
LAM = 9.0
EPS = 1e-8
MARGIN = 0.2
MASK_BIAS = -40.0

# N-tiling of the 2304-wide free axis (PSUM bank = 512 fp32)
NSL = [(0, 512), (512, 512), (1024, 512), (1536, 512), (2048, 256)]

# matmul operand dtype for the big step-1 matmul and the small reductions.
# float32r = full-rate (1 cyc/col) reduced-precision fp32; float32 = 4 cyc/col.
F32R = mybir.dt.float32r
MM_BIG = F32R   # step-1 scores + V (cholesky) matmuls
MM_RED = F32R   # denominator / threshold-broadcast / w12 / w2 reduction matmuls


def _tf32_round(x):
    t = np.ascontiguousarray(x, dtype=np.float32).view(np.uint32)
    bias = np.uint32(0x0FFF) + ((t >> np.uint32(13)) & np.uint32(1))
    return ((t + bias) & np.uint32(0xFFFFE000)).view(np.float32)


def _round_for(dt_, x):
    return _tf32_round(x) if dt_ == F32R else np.asarray(x, np.float32)


def _mm(ap, dt_):
    # tiles are allocated with their matmul dtype already; no-op
    return ap


def _build_device_program():
    nc = bacc.Bacc("TRN2", target_bir_lowering=False, debug=False)

    imT = nc.dram_tensor("imT", [D, BP], MM_BIG, kind="ExternalInput")
    sT = nc.dram_tensor("sT", [NPAIR, D, 128], MM_BIG, kind="ExternalInput")
    cch = nc.dram_tensor("cch", [NPAIR, 128, 128], MM_BIG, kind="ExternalInput")
    biasM = nc.dram_tensor("biasM", [128, NPAIR], F32, kind="ExternalInput")
    invl = nc.dram_tensor("invl", [2, NPAIR], F32, kind="ExternalInput")
    onesM2d = nc.dram_tensor("onesM2", [128, 2], MM_RED, kind="ExternalInput")
    bcast2d = nc.dram_tensor("bcast2", [2, 128], MM_RED, kind="ExternalInput")
    w12o = nc.dram_tensor("w12o", [TLOC, BP], F32, kind="ExternalOutput")
    w2o = nc.dram_tensor("w2o", [TLOC, BP], F32, kind="ExternalOutput")

    with tile.TileContext(nc) as tc:
        with ExitStack() as ctx:
            _body(ctx, tc, imT, sT, cch, biasM, invl, onesM2d, bcast2d, w12o, w2o)
    nc.compile()
    return nc


def _body(ctx, tc, imT, sT, cch, biasM, invl, onesM2d, bcast2d, w12o, w2o):
    nc = tc.nc
    consts = ctx.enter_context(tc.tile_pool(name="consts", bufs=1))
    imtp = ctx.enter_context(tc.tile_pool(name="imtp", bufs=KCH))
    stp = ctx.enter_context(tc.tile_pool(name="stp", bufs=2))
    cchp = ctx.enter_context(tc.tile_pool(name="cchp", bufs=2))
    big = ctx.enter_context(tc.tile_pool(name="big", bufs=2))
    scr = ctx.enter_context(tc.tile_pool(name="scr", bufs=2))
    small = ctx.enter_context(tc.tile_pool(name="small", bufs=3))
    rows = ctx.enter_context(tc.tile_pool(name="rows", bufs=2))
    rawp = ctx.enter_context(tc.tile_pool(name="rawp", bufs=2, space="PSUM"))
    colp = ctx.enter_context(tc.tile_pool(name="colp", bufs=2, space="PSUM"))
    rowp = ctx.enter_context(tc.tile_pool(name="rowp", bufs=2, space="PSUM"))

    # --- constants ---
    bias_sb = consts.tile([128, NPAIR], F32)
    nc.sync.dma_start(bias_sb, biasM.ap())
    invl_sb = consts.tile([2, NPAIR], F32)
    nc.sync.dma_start(invl_sb, invl.ap())

    # ones over caption-row blocks: lhsT for the partition-axis reductions
    onesM2 = consts.tile([128, 2], MM_RED)
    nc.sync.dma_start(onesM2, onesM2d.ap())
    # 2->128 partition broadcast lhsT (row r of T to partition half r)
    bcast2 = consts.tile([2, 128], MM_RED)
    nc.sync.dma_start(bcast2, bcast2d.ap())

    # --- image features, pre-transposed to [D, BP], as 8 K-chunks ---
    imt = []
    imT_r = imT.ap().rearrange("(k p) n -> k p n", p=128)
    for k in range(KCH):
        t = imtp.tile([128, BP], MM_BIG, name=f"imt{k}", tag="imt")
        nc.sync.dma_start(t, imT_r[k])
        imt.append(t)

    for pr in range(NPAIR):
        # caption-pair inputs: s^T padded into a [D, 128] block layout
        sT_sb = stp.tile([128, KCH, 128], MM_BIG, name="sT_sb")
        nc.sync.dma_start(sT_sb, sT.ap()[pr].rearrange("(k q) m -> q k m", q=128))
        cch_sb = cchp.tile([128, 128], MM_BIG, name="cch_sb")
        nc.sync.dma_start(cch_sb, cch.ap()[pr])

        # --- step 1: raw[l, bp] = s_t . im  (both captions at once) ---
        raw_sb = big.tile([128, BP], F32, name="raw_sb")
        for (n0, nn) in NSL:
            ps = rawp.tile([128, 512], F32, name="rawps", tag="rawps")[:, :nn]
            for k in range(KCH):
                nc.tensor.matmul(
                    ps,
                    lhsT=_mm(sT_sb[:, k, :], MM_BIG),
                    rhs=_mm(imt[k][:, n0:n0 + nn], MM_BIG),
                    start=(k == 0),
                    stop=(k == KCH - 1),
                )
            nc.scalar.copy(raw_sb[:, n0:n0 + nn], ps)

        # --- leaky relu: leak = max(0.1*raw, raw) (one fused op) ---
        leak = big.tile([128, BP], F32, name="leak")
        nc.vector.scalar_tensor_tensor(
            leak, raw_sb, 0.1, raw_sb, op0=OP.mult, op1=OP.max
        )

        # --- l2 norm over the 36-region groups ---
        xsq = scr.tile([128, BP], F32, name="xsq", tag="scratch")
        nc.scalar.activation(xsq, leak, AF.Square)
        nsum = small.tile([128, B], F32, name="nsum")
        nc.vector.reduce_sum(
            nsum,
            xsq.rearrange("p (b q) -> p b q", q=P),
            axis=mybir.AxisListType.X,
        )
        rt = small.tile([128, B], F32, name="rt")
        nc.scalar.activation(rt, nsum, AF.Sqrt)
        nc.vector.tensor_scalar_add(rt, rt, EPS)
        fac = small.tile([128, B], F32, name="fac")
        nc.vector.reciprocal(fac, rt)
        nc.vector.tensor_scalar_mul(fac, fac, LAM)
        # xhat*lam, in place over leak
        leak3 = leak.rearrange("p (b q) -> p b q", q=P)
        nc.vector.tensor_mul(leak3, leak3, fac[:, :, None].to_broadcast([128, B, P]))

        # --- E = exp(lam*xhat + mask_bias) ---
        E = big.tile([128, BP], MM_RED, name="E")
        nc.scalar.activation(E, leak, AF.Exp, bias=bias_sb[:, pr:pr + 1], scale=1.0)

        # --- softmax denominator rows (per caption) via ones-matmul ---
        denom = rows.tile([2, BP], MM_RED, name="denom", tag="rows")
        for (n0, nn) in NSL:
            rp = rowp.tile([2, 512], F32, name="rowps", tag="rowps")[:, :nn]
            nc.tensor.matmul(
                rp, lhsT=onesM2, rhs=_mm(E[:, n0:n0 + nn], MM_RED)
            )
            nc.vector.tensor_copy(denom[:, n0:n0 + nn], rp)
        # threshold rows T = denom / cap_len (in place)
        nc.vector.tensor_scalar_mul(denom, denom, invl_sb[:, pr:pr + 1])

        # --- threshold: G = E * (E > T_broadcast) ---
        Gm = scr.tile([128, BP], F32, name="Gm", tag="scratch")
        for (n0, nn) in NSL:
            tb = colp.tile([128, 512], F32, name="colps", tag="colps")[:, :nn]
            nc.tensor.matmul(
                tb, lhsT=bcast2, rhs=_mm(denom[:, n0:n0 + nn], MM_RED)
            )
            nc.vector.tensor_tensor(
                Gm[:, n0:n0 + nn], E[:, n0:n0 + nn], tb, op=OP.is_gt
            )
        G = big.tile([128, BP], MM_BIG, name="G")
        nc.vector.tensor_mul(G, E, Gm)

        # --- w12 = sum_l G * raw ---
        prod = big.tile([128, BP], MM_RED, name="prod", tag="leak")
        nc.gpsimd.tensor_mul(prod, G, raw_sb)
        w12sb = rows.tile([2, BP], F32, name="w12sb", tag="rows")
        for (n0, nn) in NSL:
            rp = rowp.tile([2, 512], F32, name="rowps", tag="rowps")[:, :nn]
            nc.tensor.matmul(
                rp, lhsT=onesM2, rhs=_mm(prod[:, n0:n0 + nn], MM_RED)
            )
            nc.vector.tensor_copy(w12sb[:, n0:n0 + nn], rp)
        nc.sync.dma_start(w12o.ap()[2 * pr:2 * pr + 2, :], w12sb)

        # --- w2sq = sum_l (C^T G)^2 ---
        sqV = scr.tile([128, BP], MM_RED, name="sqV", tag="scratch")
        for (n0, nn) in NSL:
            vp = colp.tile([128, 512], F32, name="colps", tag="colps")[:, :nn]
            nc.tensor.matmul(
                vp, lhsT=cch_sb, rhs=_mm(G[:, n0:n0 + nn], MM_BIG)
            )
            nc.scalar.activation(sqV[:, n0:n0 + nn], vp, AF.Square)
        w2sb = rows.tile([2, BP], F32, name="w2sb", tag="rows")
        for (n0, nn) in NSL:
            rp = rowp.tile([2, 512], F32, name="rowps", tag="rowps")[:, :nn]
            nc.tensor.matmul(
                rp, lhsT=onesM2, rhs=_mm(sqV[:, n0:n0 + nn], MM_RED)
            )
            nc.vector.tensor_copy(w2sb[:, n0:n0 + nn], rp)
        nc.sync.dma_start(w2o.ap()[2 * pr:2 * pr + 2, :], w2sb)


_ONESM2 = np.zeros((128, 2), np.float32)
_ONESM2[0:L, 0] = 1.0
_ONESM2[64:64 + L, 1] = 1.0
_BCAST2 = np.zeros((2, 128), np.float32)
_BCAST2[0, 0:64] = 1.0
_BCAST2[1, 64:128] = 1.0

_CACHE = {}


def _get_program():
    if "nc" not in _CACHE:
        _CACHE["nc"] = _build_device_program()
    return _CACHE["nc"]


def _host_inputs(im, s, cl):
    """Build per-core in_maps (host-side sharding + layout prep)."""
    imT = _round_for(MM_BIG, np.ascontiguousarray(im.reshape(B * P, D).T))
    in_maps = []
    for c in range(NCORES):
        s_loc = s[c * TLOC:(c + 1) * TLOC]
        cl_loc = cl[c * TLOC:(c + 1) * TLOC]
        sT = np.zeros((NPAIR, D, 128), np.float32)
        cc = np.zeros((NPAIR, 128, 128), np.float32)
        bm = np.full((128, NPAIR), MASK_BIAS, np.float32)
        iv = np.zeros((2, NPAIR), np.float32)
        for pr in range(NPAIR):
            for j in (0, 1):
                t = 2 * pr + j
                off = 64 * j
                st = s_loc[t]
                sT[pr, :, off:off + L] = st.T
                K = (st.astype(np.float64) @ st.astype(np.float64).T)
                C = np.linalg.cholesky(K).astype(np.float32)
                cc[pr, off:off + L, off:off + L] = C
                bm[off:off + L, pr] = np.where(
                    np.arange(L) < cl_loc[t], 0.0, MASK_BIAS
                ).astype(np.float32)
                iv[j, pr] = np.float32(1.0 / float(cl_loc[t]))
        in_maps.append({
            "imT": imT, "sT": _round_for(MM_BIG, sT), "cch": _round_for(MM_BIG, cc),
            "biasM": bm, "invl": iv, "onesM2": _ONESM2, "bcast2": _BCAST2,
        })
    return in_maps


def _host_tail(im, w12, w2sq):
    imf = im.reshape(B * P, D)
    w1 = np.sqrt(np.sum(imf * imf, axis=1, dtype=np.float32))
    w2 = np.sqrt(np.maximum(w2sq, 0.0))
    cos = w12 / np.maximum(w1[None, :] * w2, np.float32(EPS))
    cosr = cos.reshape(B, B, P)
    scores = np.sort(cosr, axis=-1)[..., P // 3:].mean(axis=-1, dtype=np.float32)
    d = np.diag(scores).copy()
    cs = np.maximum(np.float32(MARGIN) + scores - d[:, None], 0.0)
    ci = np.maximum(np.float32(MARGIN) + scores - d[None, :], 0.0)
    np.fill_diagonal(cs, 0.0)
    np.fill_diagonal(ci, 0.0)
    loss = cs.max(axis=1).sum(dtype=np.float32) + ci.max(axis=0).sum(dtype=np.float32)
    return np.asarray(loss, dtype=np.float32)


def kernel(im, s, cap_lens, _profile=False):
    im = np.ascontiguousarray(np.asarray(im, dtype=np.float32))
    s = np.ascontiguousarray(np.asarray(s, dtype=np.float32))
    cl = np.asarray(cap_lens).astype(np.int64)

    nc = _get_program()
    in_maps = _host_inputs(im, s, cl)
    kw = dict(trace=True) if _profile else {}
    res = bass_utils.run_bass_kernel_spmd(
        nc, in_maps, core_ids=list(range(NCORES)), **kw
    )
    w12 = np.concatenate([res.results[c]["w12o"] for c in range(NCORES)], axis=0)
    w2sq = np.concatenate([res.results[c]["w2o"] for c in range(NCORES)], axis=0)
    out = _host_tail(im, w12, w2sq)
    if _profile:
        return out, res
    return out


# revision 22
# speedup vs baseline: 2.3802x; 1.4767x over previous
"""Trainium2 Bass kernel for nn_ContrastiveLoss (sparse cross-attention t2i loss).

Strategy (sharding_hint): shard the caption (txt) batch axis across the 8
cores — 8 captions per core, processed in 4 pairs of 2 (two 50-word captions
packed into the 128-partition dim at offsets 0 / 64). The image batch
(64 imgs x 36 regions = 2304 "bp" columns) is replicated.

Math reformulation (avoids materializing the (Bt,Bi,P,D) weighted context):
  raw[l,bp]  = s_t[l] . im[bp]                    (PE, D=1024 contraction)
  leak       = max(raw, 0.1*raw)                  (LeakyReLU 0.1)
  xhat       = leak / (||leak||_2 over p + eps)   (region-axis l2norm)
  E          = exp(9*xhat + biasmask)             (word mask via -40 bias)
  denom[bp]  = sum_l E  (ones-matmul on PE)       (softmax denominator)
  G          = E * (E > denom/cap_len)            (threshold sparsify,
                                                   unnormalized attention)
  w12[bp]    = sum_l G*raw                        (= denom * <im, wc>)
  w2sq[bp]   = ||C^T G||^2, K=s s^T=C C^T (chol)  (= denom^2 * ||wc||^2)
  cos        = w12 / max(w1 * sqrt(w2sq), eps)    (denom scale cancels)
The diagonal-margin loss over the gathered (64,64) score matrix is computed
on the host (a few thousand flops).
"""

import numpy as np
from contextlib import ExitStack

import concourse.bass as bass
import concourse.bacc as bacc
import concourse.tile as tile
from concourse import mybir, bass_utils

F32 = mybir.dt.float32
AF = mybir.ActivationFunctionType
OP = mybir.AluOpType

B = 64          # batch (captions == images)
P = 36          # regions per image
D = 1024        # feature dim
L = 50          # padded words per caption
BP = B * P      # 2304 score columns
NCORES = 8
TLOC = B // NCORES   # 8 captions per core
NPAIR = TLOC // 2    # 4 caption pairs per core
KCH = D // 128       # 8 contraction chunks
LAM = 9.0
EPS = 1e-8
MARGIN = 0.2
MASK_BIAS = -40.0

# N-tiling of the 2304-wide free axis (PSUM bank = 512 fp32)
NSL = [(0, 512), (512, 512), (1024, 512), (1536, 512), (2048, 256)]

# matmul operand dtype for the big step-1 matmul and the small reductions.
# float32r = full-rate (1 cyc/col) reduced-precision fp32; float32 = 4 cyc/col.
F32R = mybir.dt.float32r
MM_BIG = F32R   # step-1 scores + V (cholesky) matmuls
MM_RED = F32R   # denominator / threshold-broadcast / w12 / w2 reduction matmuls


def _tf32_round(x):
    t = np.ascontiguousarray(x, dtype=np.float32).view(np.uint32)
    bias = np.uint32(0x0FFF) + ((t >> np.uint32(13)) & np.uint32(1))
    return ((t + bias) & np.uint32(0xFFFFE000)).view(np.float32)


def _round_for(dt_, x):
    return _tf32_round(x) if dt_ == F32R else np.asarray(x, np.float32)


def _mm(ap, dt_):
    # tiles are allocated with their matmul dtype already; no-op
    return ap


def _build_device_program():
    nc = bacc.Bacc("TRN2", target_bir_lowering=False, debug=False)

    imT = nc.dram_tensor("imT", [D, BP], MM_BIG, kind="ExternalInput")
    sT = nc.dram_tensor("sT", [NPAIR, D, 128], MM_BIG, kind="ExternalInput")
    cch = nc.dram_tensor("cch", [NPAIR, 128, 128], MM_BIG, kind="ExternalInput")
    biasM = nc.dram_tensor("biasM", [128, NPAIR], F32, kind="ExternalInput")
    invl = nc.dram_tensor("invl", [2, NPAIR], F32, kind="ExternalInput")
    onesM2d = nc.dram_tensor("onesM2", [128, 2], MM_RED, kind="ExternalInput")
    bcast2d = nc.dram_tensor("bcast2", [2, 128], MM_RED, kind="ExternalInput")
    w12o = nc.dram_tensor("w12o", [TLOC, BP], F32, kind="ExternalOutput")
    w2o = nc.dram_tensor("w2o", [TLOC, BP], F32, kind="ExternalOutput")

    with tile.TileContext(nc) as tc:
        with ExitStack() as ctx:
            _body(ctx, tc, imT, sT, cch, biasM, invl, onesM2d, bcast2d, w12o, w2o)
    nc.compile()
    return nc


def _body(ctx, tc, imT, sT, cch, biasM, invl, onesM2d, bcast2d, w12o, w2o):
    nc = tc.nc
    NS = 288                  # matmul N-tile (psum bank-safe, f32r full rate)
    ES = 576                  # elementwise slice (16 groups of 36)
    NES = BP // ES            # 4 elementwise slices
    GRP = ES // P             # 16 norm groups per slice

    consts = ctx.enter_context(tc.tile_pool(name="consts", bufs=1))
    imtp = ctx.enter_context(tc.tile_pool(name="imtp", bufs=BP // NS))
    stp = ctx.enter_context(tc.tile_pool(name="stp", bufs=2))
    cchp = ctx.enter_context(tc.tile_pool(name="cchp", bufs=2))
    big = ctx.enter_context(tc.tile_pool(name="big", bufs=2))
    small = ctx.enter_context(tc.tile_pool(name="small", bufs=3))
    rows = ctx.enter_context(tc.tile_pool(name="rows", bufs=2))
    rawp = ctx.enter_context(tc.tile_pool(name="rawp", bufs=2, space="PSUM"))
    colp = ctx.enter_context(tc.tile_pool(name="colp", bufs=2, space="PSUM"))
    rowp = ctx.enter_context(tc.tile_pool(name="rowp", bufs=1, space="PSUM"))

    # --- constants ---
    bias_sb = consts.tile([128, NPAIR], F32)
    nc.sync.dma_start(bias_sb, biasM.ap())
    invl_sb = consts.tile([2, NPAIR], F32)
    nc.sync.dma_start(invl_sb, invl.ap())
    onesM2 = consts.tile([128, 2], MM_RED)
    nc.sync.dma_start(onesM2, onesM2d.ap())
    bcast2 = consts.tile([2, 128], MM_RED)
    nc.sync.dma_start(bcast2, bcast2d.ap())
    magic = consts.tile([128, B], mybir.dt.int32)
    nc.vector.memset(magic, 0x5F3759DF)

    # --- image features [D, BP] loaded as N-slices of [128, KCH, NS] ---
    imts = []
    imT_r = imT.ap().rearrange("(k p) n -> p k n", p=128)
    for n in range(BP // NS):
        t = imtp.tile([128, KCH, NS], MM_BIG, name=f"imt{n}", tag="imt")
        nc.sync.dma_start(t, imT_r[:, :, n * NS:(n + 1) * NS])
        imts.append(t)

    def rsqrt_nr(fac, nsum, ints):
        """fac = 1/sqrt(nsum) via magic-seed + 2 Newton iterations (DVE)."""
        # clamp: pad rows have nsum == 0; keeps the seed finite (x*0 = 0 later)
        nc.vector.tensor_scalar_max(nsum, nsum, 1e-12)
        nc.vector.tensor_scalar(
            ints, nsum.bitcast(mybir.dt.int32), 1, None,
            op0=OP.logical_shift_right,
        )
        nc.vector.tensor_tensor(
            fac.bitcast(mybir.dt.int32), magic, ints, op=OP.subtract
        )
        a = small.tile([128, B], F32, name="nr_a")
        for _ in range(2):
            nc.vector.tensor_mul(a, fac, fac)
            nc.vector.tensor_mul(a, a, nsum)
            nc.vector.tensor_scalar(a, a, -0.5, 1.5, op0=OP.mult, op1=OP.add)
            nc.vector.tensor_mul(fac, fac, a)

    state = {}

    def emit_A(pr):
        """scores + leaky + norm stats for pair pr (slice-pipelined)."""
        sT_sb = stp.tile([128, KCH, 128], MM_BIG, name="sT_sb")
        nc.gpsimd.dma_start(sT_sb, sT.ap()[pr].rearrange("(k q) m -> q k m", q=128))
        cch_sb = cchp.tile([128, 128], MM_BIG, name="cch_sb")
        nc.gpsimd.dma_start(cch_sb, cch.ap()[pr])
        nsum = small.tile([128, B], F32, name="nsum")
        raws, leaks = [], []
        for s in range(NES):
            raw = big.tile([128, ES], F32, name="raw", tag="raw", bufs=2 * NES)
            raws.append(raw)
            for h, n in enumerate((2 * s, 2 * s + 1)):
                ps = rawp.tile([128, NS], F32, name="rawps", tag="rawps")
                for k in range(KCH):
                    nc.tensor.matmul(
                        ps,
                        lhsT=sT_sb[:, k, :],
                        rhs=imts[n][:, k, :],
                        start=(k == 0),
                        stop=(k == KCH - 1),
                    )
                nc.scalar.copy(raw[:, h * NS:(h + 1) * NS], ps)
            # LeakyReLU(0.1) on ACT (parametric relu; shares the exp act table)
            leak = big.tile([128, ES], F32, name="leak", tag="leak", bufs=8)
            leaks.append(leak)
            nc.scalar.activation(leak, raw, AF.Prelu, alpha=0.1)
            sq = big.tile([128, ES], F32, name="sq", tag="scr", bufs=8)
            nc.gpsimd.tensor_mul(sq, leak, leak)
            nc.vector.reduce_sum(
                nsum[:, s * GRP:(s + 1) * GRP],
                sq.rearrange("p (b q) -> p b q", q=P),
                axis=mybir.AxisListType.X,
            )
        state[pr] = (raws, leaks, nsum, cch_sb)

    def emit_B(pr):
        """softmax denominator, threshold, w12/w2 reductions for pair pr."""
        raws, leaks, nsum, cch_sb = state.pop(pr)
        fac = small.tile([128, B], F32, name="fac")
        ints = small.tile([128, B], mybir.dt.int32, name="ints")
        rsqrt_nr(fac, nsum, ints)

        denom = rows.tile([2, BP], MM_RED, name="denom", tag="denom")
        w12sb = rows.tile([2, BP], F32, name="w12sb", tag="w12sb", bufs=1)
        w2sb = rows.tile([2, BP], F32, name="w2sb", tag="w2sb", bufs=1)

        Es, Gs, prods = [], [], []
        for s in range(NES):
            sl = slice(s * ES, (s + 1) * ES)
            leak = leaks[s]
            l3 = leak.rearrange("p (b q) -> p b q", q=P)
            nc.vector.tensor_mul(
                l3, l3,
                fac[:, s * GRP:(s + 1) * GRP, None].to_broadcast([128, GRP, P]),
            )
            E = big.tile([128, ES], MM_RED, name="E", tag="E", bufs=6)
            Es.append(E)
            nc.scalar.activation(
                E, leak, AF.Exp, bias=bias_sb[:, pr:pr + 1], scale=LAM
            )
            # denominator rows for both N-halves, one psum pair-tile
            rp = rowp.tile([2, 2, 512], F32, name="rowps", tag="rowps")
            for h in range(2):
                nc.tensor.matmul(
                    rp[:, h, :NS], lhsT=onesM2, rhs=E[:, h * NS:(h + 1) * NS]
                )
            # T = denom/cap_len folded into the psum->sbuf copy
            nc.vector.tensor_scalar_mul(
                denom[:, sl].rearrange("r (h n) -> r h n", h=2),
                rp[:, :, :NS],
                invl_sb[:, pr:pr + 1],
            )
            # threshold broadcast + compare (both halves, one cmp)
            tb = colp.tile([128, 2, 512], F32, name="colps", tag="colps")
            for h, n in enumerate((2 * s, 2 * s + 1)):
                nc.tensor.matmul(
                    tb[:, h, :NS], lhsT=bcast2, rhs=denom[:, n * NS:(n + 1) * NS]
                )
            Gm = big.tile([128, ES], F32, name="Gm", tag="scr", bufs=8)
            nc.vector.tensor_tensor(
                Gm.rearrange("p (h n) -> p h n", h=2),
                E.rearrange("p (h n) -> p h n", h=2),
                tb[:, :, :NS],
                op=OP.is_gt,
            )
            G = big.tile([128, ES], MM_BIG, name="G", tag="G", bufs=8)
            Gs.append(G)
            nc.vector.tensor_mul(G, E, Gm)
            prod = big.tile([128, ES], MM_RED, name="prod", tag="leak", bufs=8)
            prods.append(prod)
            nc.gpsimd.tensor_mul(prod, G, raws[s])

        sqVs = []
        for s in range(NES):
            sl = slice(s * ES, (s + 1) * ES)
            # w12 rows
            rp = rowp.tile([2, 2, 512], F32, name="rowps", tag="rowps")
            for h in range(2):
                nc.tensor.matmul(
                    rp[:, h, :NS], lhsT=onesM2, rhs=prods[s][:, h * NS:(h + 1) * NS]
                )
            nc.scalar.copy(
                w12sb[:, sl].rearrange("r (h n) -> r h n", h=2), rp[:, :, :NS]
            )
            # V = C^T G, squared on ACT
            vp = colp.tile([128, 2, 512], F32, name="colps", tag="colps")
            for h in range(2):
                nc.tensor.matmul(
                    vp[:, h, :NS], lhsT=cch_sb, rhs=Gs[s][:, h * NS:(h + 1) * NS]
                )
            sqV = big.tile([128, ES], MM_RED, name="sqV", tag="scr", bufs=8)
            sqVs.append(sqV)
            nc.scalar.activation(
                sqV.rearrange("p (h n) -> p h n", h=2), vp[:, :, :NS], AF.Square
            )
        for s in range(NES):
            sl = slice(s * ES, (s + 1) * ES)
            rp = rowp.tile([2, 2, 512], F32, name="rowps", tag="rowps")
            for h in range(2):
                nc.tensor.matmul(
                    rp[:, h, :NS], lhsT=onesM2, rhs=sqVs[s][:, h * NS:(h + 1) * NS]
                )
            nc.vector.tensor_copy(
                w2sb[:, sl].rearrange("r (h n) -> r h n", h=2), rp[:, :, :NS]
            )

        nc.sync.dma_start(w12o.ap()[2 * pr:2 * pr + 2, :], w12sb)
        nc.sync.dma_start(w2o.ap()[2 * pr:2 * pr + 2, :], w2sb)

    # software-pipelined emission: pair pr+1's phase A ahead of pair pr's phase B
    emit_A(0)
    for pr in range(NPAIR):
        if pr + 1 < NPAIR:
            emit_A(pr + 1)
        emit_B(pr)


_ONESM2 = np.zeros((128, 2), np.float32)
_ONESM2[0:L, 0] = 1.0
_ONESM2[64:64 + L, 1] = 1.0
_BCAST2 = np.zeros((2, 128), np.float32)
_BCAST2[0, 0:64] = 1.0
_BCAST2[1, 64:128] = 1.0

_CACHE = {}


def _get_program():
    if "nc" not in _CACHE:
        _CACHE["nc"] = _build_device_program()
    return _CACHE["nc"]


def _host_inputs(im, s, cl):
    """Build per-core in_maps (host-side sharding + layout prep)."""
    imT = _round_for(MM_BIG, np.ascontiguousarray(im.reshape(B * P, D).T))
    in_maps = []
    for c in range(NCORES):
        s_loc = s[c * TLOC:(c + 1) * TLOC]
        cl_loc = cl[c * TLOC:(c + 1) * TLOC]
        sT = np.zeros((NPAIR, D, 128), np.float32)
        cc = np.zeros((NPAIR, 128, 128), np.float32)
        bm = np.full((128, NPAIR), MASK_BIAS, np.float32)
        iv = np.zeros((2, NPAIR), np.float32)
        for pr in range(NPAIR):
            for j in (0, 1):
                t = 2 * pr + j
                off = 64 * j
                st = s_loc[t]
                sT[pr, :, off:off + L] = st.T
                K = (st.astype(np.float64) @ st.astype(np.float64).T)
                C = np.linalg.cholesky(K).astype(np.float32)
                cc[pr, off:off + L, off:off + L] = C
                bm[off:off + L, pr] = np.where(
                    np.arange(L) < cl_loc[t], 0.0, MASK_BIAS
                ).astype(np.float32)
                iv[j, pr] = np.float32(1.0 / float(cl_loc[t]))
        in_maps.append({
            "imT": imT, "sT": _round_for(MM_BIG, sT), "cch": _round_for(MM_BIG, cc),
            "biasM": bm, "invl": iv, "onesM2": _ONESM2, "bcast2": _BCAST2,
        })
    return in_maps


def _host_tail(im, w12, w2sq):
    imf = im.reshape(B * P, D)
    w1 = np.sqrt(np.sum(imf * imf, axis=1, dtype=np.float32))
    w2 = np.sqrt(np.maximum(w2sq, 0.0))
    cos = w12 / np.maximum(w1[None, :] * w2, np.float32(EPS))
    cosr = cos.reshape(B, B, P)
    scores = np.sort(cosr, axis=-1)[..., P // 3:].mean(axis=-1, dtype=np.float32)
    d = np.diag(scores).copy()
    cs = np.maximum(np.float32(MARGIN) + scores - d[:, None], 0.0)
    ci = np.maximum(np.float32(MARGIN) + scores - d[None, :], 0.0)
    np.fill_diagonal(cs, 0.0)
    np.fill_diagonal(ci, 0.0)
    loss = cs.max(axis=1).sum(dtype=np.float32) + ci.max(axis=0).sum(dtype=np.float32)
    return np.asarray(loss, dtype=np.float32)


def kernel(im, s, cap_lens, _profile=False):
    im = np.ascontiguousarray(np.asarray(im, dtype=np.float32))
    s = np.ascontiguousarray(np.asarray(s, dtype=np.float32))
    cl = np.asarray(cap_lens).astype(np.int64)

    nc = _get_program()
    in_maps = _host_inputs(im, s, cl)
    kw = dict(trace=True) if _profile else {}
    res = bass_utils.run_bass_kernel_spmd(
        nc, in_maps, core_ids=list(range(NCORES)), **kw
    )
    w12 = np.concatenate([res.results[c]["w12o"] for c in range(NCORES)], axis=0)
    w2sq = np.concatenate([res.results[c]["w2o"] for c in range(NCORES)], axis=0)
    out = _host_tail(im, w12, w2sq)
    if _profile:
        return out, res
    return out


# revision 31
# speedup vs baseline: 2.5960x; 1.0906x over previous
"""Trainium2 Bass kernel for nn_ContrastiveLoss (sparse cross-attention t2i loss).

Strategy (sharding_hint): shard the caption (txt) batch axis across the 8
cores — 8 captions per core, processed in 4 pairs of 2 (two 50-word captions
packed into the 128-partition dim at offsets 0 / 64). The image batch
(64 imgs x 36 regions = 2304 "bp" columns) is replicated.

Math reformulation (avoids materializing the (Bt,Bi,P,D) weighted context):
  raw[l,bp]  = s_t[l] . im[bp]                    (PE, D=1024 contraction)
  leak       = max(raw, 0.1*raw)                  (LeakyReLU 0.1)
  xhat       = leak / (||leak||_2 over p + eps)   (region-axis l2norm)
  E          = exp(9*xhat + biasmask)             (word mask via -40 bias)
  denom[bp]  = sum_l E  (ones-matmul on PE)       (softmax denominator)
  G          = E * (E > denom/cap_len)            (threshold sparsify,
                                                   unnormalized attention)
  w12[bp]    = sum_l G*raw                        (= denom * <im, wc>)
  w2sq[bp]   = ||C^T G||^2, K=s s^T=C C^T (chol)  (= denom^2 * ||wc||^2)
  cos        = w12 / max(w1 * sqrt(w2sq), eps)    (denom scale cancels)
The diagonal-margin loss over the gathered (64,64) score matrix is computed
on the host (a few thousand flops).
"""

import numpy as np
from contextlib import ExitStack

import concourse.bass as bass
import concourse.bacc as bacc
import concourse.tile as tile
from concourse import mybir, bass_utils

F32 = mybir.dt.float32
AF = mybir.ActivationFunctionType
OP = mybir.AluOpType

B = 64          # batch (captions == images)
P = 36          # regions per image
D = 1024        # feature dim
L = 50          # padded words per caption
BP = B * P      # 2304 score columns
NCORES = 8
TLOC = B // NCORES   # 8 captions per core
NPAIR = TLOC // 2    # 4 caption pairs per core
KCH = D // 128       # 8 contraction chunks
LAM = 9.0
EPS = 1e-8
MARGIN = 0.2
MASK_BIAS = -40.0

# N-tiling of the 2304-wide free axis (PSUM bank = 512 fp32)
NSL = [(0, 512), (512, 512), (1024, 512), (1536, 512), (2048, 256)]

# matmul operand dtype for the big step-1 matmul and the small reductions.
# float32r = full-rate (1 cyc/col) reduced-precision fp32; float32 = 4 cyc/col.
F32R = mybir.dt.float32r
MM_BIG = F32R   # step-1 scores + V (cholesky) matmuls
MM_RED = F32R   # denominator / threshold-broadcast / w12 / w2 reduction matmuls


def _tf32_round(x):
    t = np.ascontiguousarray(x, dtype=np.float32).view(np.uint32)
    bias = np.uint32(0x0FFF) + ((t >> np.uint32(13)) & np.uint32(1))
    return ((t + bias) & np.uint32(0xFFFFE000)).view(np.float32)


def _round_for(dt_, x):
    return _tf32_round(x) if dt_ == F32R else np.asarray(x, np.float32)


def _mm(ap, dt_):
    # tiles are allocated with their matmul dtype already; no-op
    return ap


def _build_device_program():
    nc = bacc.Bacc("TRN2", target_bir_lowering=False, debug=False)

    imT = nc.dram_tensor("imT", [D, BP], MM_BIG, kind="ExternalInput")
    sT = nc.dram_tensor("sT", [NPAIR, D, 128], MM_BIG, kind="ExternalInput")
    cch = nc.dram_tensor("cch", [NPAIR, 128, 128], MM_BIG, kind="ExternalInput")
    biasM = nc.dram_tensor("biasM", [128, NPAIR], F32, kind="ExternalInput")
    invl = nc.dram_tensor("invl", [2, NPAIR], F32, kind="ExternalInput")
    onesM2d = nc.dram_tensor("onesM2", [128, 2], MM_RED, kind="ExternalInput")
    bcast2d = nc.dram_tensor("bcast2", [2, 128], MM_RED, kind="ExternalInput")
    w12o = nc.dram_tensor("w12o", [TLOC, BP], F32, kind="ExternalOutput")
    w2o = nc.dram_tensor("w2o", [TLOC, BP], F32, kind="ExternalOutput")

    with tile.TileContext(nc) as tc:
        with ExitStack() as ctx:
            _body(ctx, tc, imT, sT, cch, biasM, invl, onesM2d, bcast2d, w12o, w2o)
    nc.compile()
    return nc


def _body(ctx, tc, imT, sT, cch, biasM, invl, onesM2d, bcast2d, w12o, w2o):
    nc = tc.nc
    NS = 288                  # matmul N-tile (psum bank-safe, f32r full rate)
    ES = 576                  # elementwise slice (16 groups of 36)
    NES = BP // ES            # 4 elementwise slices
    GRP = ES // P             # 16 norm groups per slice

    consts = ctx.enter_context(tc.tile_pool(name="consts", bufs=1))
    imtp = ctx.enter_context(tc.tile_pool(name="imtp", bufs=BP // NS))
    stp = ctx.enter_context(tc.tile_pool(name="stp", bufs=2))
    cchp = ctx.enter_context(tc.tile_pool(name="cchp", bufs=2))
    big = ctx.enter_context(tc.tile_pool(name="big", bufs=2))
    small = ctx.enter_context(tc.tile_pool(name="small", bufs=3))
    rows = ctx.enter_context(tc.tile_pool(name="rows", bufs=2))
    rawp = ctx.enter_context(tc.tile_pool(name="rawp", bufs=2, space="PSUM"))
    colp = ctx.enter_context(tc.tile_pool(name="colp", bufs=2, space="PSUM"))
    rowp = ctx.enter_context(tc.tile_pool(name="rowp", bufs=1, space="PSUM"))

    # --- constants on the gpsimd queue (keeps the sync queue free for imT) ---
    bias_sb = consts.tile([128, NPAIR], F32)
    nc.gpsimd.dma_start(bias_sb, biasM.ap())
    invl_sb = consts.tile([2, NPAIR], F32)
    nc.gpsimd.dma_start(invl_sb, invl.ap())
    onesM2 = consts.tile([128, 2], MM_RED)
    nc.gpsimd.dma_start(onesM2, onesM2d.ap())
    bcast2 = consts.tile([2, 128], MM_RED)
    nc.gpsimd.dma_start(bcast2, bcast2d.ap())
    magic = consts.tile([128, B], mybir.dt.int32)
    nc.vector.memset(magic, 0x5F3759DF)

    # pair-0 caption inputs ahead of the big image transfer
    first_sT = stp.tile([128, KCH, 128], MM_BIG, name="sT_sb")
    nc.sync.dma_start(first_sT, sT.ap()[0].rearrange("(k q) m -> q k m", q=128))
    first_cch = cchp.tile([128, 128], MM_BIG, name="cch_sb")
    nc.gpsimd.dma_start(first_cch, cch.ap()[0])

    # --- image features [D, BP] loaded as N-slices of [128, KCH, NS] ---
    imts = []
    imT_r = imT.ap().rearrange("(k p) n -> p k n", p=128)
    for n in range(BP // NS):
        t = imtp.tile([128, KCH, NS], MM_BIG, name=f"imt{n}", tag="imt")
        sl = imT_r[:, :, n * NS:(n + 1) * NS]
        if n == 0:
            nc.sync.dma_start(t[:, :4, :], sl[:, :4, :])
            nc.sync.dma_start(t[:, 4:, :], sl[:, 4:, :])
        else:
            nc.sync.dma_start(t, sl)
        imts.append(t)

    def rsqrt_nr(fac, nsum, ints):
        """fac = 1/sqrt(nsum) via magic-seed + 2 Newton iterations (DVE)."""
        # clamp: pad rows have nsum == 0; keeps the seed finite (x*0 = 0 later)
        nc.vector.tensor_scalar_max(nsum, nsum, 1e-12)
        nc.vector.tensor_scalar(
            ints, nsum.bitcast(mybir.dt.int32), 1, None,
            op0=OP.logical_shift_right,
        )
        nc.vector.tensor_tensor(
            fac.bitcast(mybir.dt.int32), magic, ints, op=OP.subtract
        )
        a = small.tile([128, B], F32, name="nr_a")
        for _ in range(2):
            nc.vector.tensor_mul(a, fac, fac)
            nc.vector.tensor_mul(a, a, nsum)
            nc.vector.tensor_scalar(a, a, -0.5, 1.5, op0=OP.mult, op1=OP.add)
            nc.vector.tensor_mul(fac, fac, a)

    state = {}

    def emit_A(pr):
        """scores + leaky + norm stats for pair pr (slice-pipelined)."""
        if pr == 0:
            sT_sb, cch_sb = first_sT, first_cch
        else:
            sT_sb = stp.tile([128, KCH, 128], MM_BIG, name="sT_sb")
            nc.sync.dma_start(sT_sb, sT.ap()[pr].rearrange("(k q) m -> q k m", q=128))
            cch_sb = cchp.tile([128, 128], MM_BIG, name="cch_sb")
            nc.sync.dma_start(cch_sb, cch.ap()[pr])
        nsum = small.tile([128, B], F32, name="nsum")
        raws, leaks = [], []
        for s in range(NES):
            raw = big.tile([128, ES], F32, name="raw", tag="raw", bufs=2 * NES)
            raws.append(raw)
            for h, n in enumerate((2 * s, 2 * s + 1)):
                ps = rawp.tile([128, NS], F32, name="rawps", tag="rawps")
                for k in range(KCH):
                    nc.tensor.matmul(
                        ps,
                        lhsT=sT_sb[:, k, :],
                        rhs=imts[n][:, k, :],
                        start=(k == 0),
                        stop=(k == KCH - 1),
                    )
                nc.scalar.copy(raw[:, h * NS:(h + 1) * NS], ps)
            # LeakyReLU(0.1) on ACT (parametric relu; shares the exp act table)
            leak = big.tile([128, ES], F32, name="leak", tag="leak", bufs=8)
            leaks.append(leak)
            nc.scalar.activation(leak, raw, AF.Prelu, alpha=0.1)
            sq = big.tile([128, ES], F32, name="sq", tag="scr", bufs=8)
            nc.gpsimd.tensor_mul(sq, leak, leak)
            nc.vector.reduce_sum(
                nsum[:, s * GRP:(s + 1) * GRP],
                sq.rearrange("p (b q) -> p b q", q=P),
                axis=mybir.AxisListType.X,
            )
        state[pr] = (raws, leaks, nsum, cch_sb)

    def emit_B(pr):
        """softmax denominator, threshold, w12/w2 reductions for pair pr."""
        raws, leaks, nsum, cch_sb = state.pop(pr)
        fac = small.tile([128, B], F32, name="fac")
        ints = small.tile([128, B], mybir.dt.int32, name="ints")
        rsqrt_nr(fac, nsum, ints)

        denom = rows.tile([2, BP], MM_RED, name="denom", tag="denom")
        w12sb = rows.tile([2, BP], F32, name="w12sb", tag="w12sb", bufs=1)
        w2sb = rows.tile([2, BP], F32, name="w2sb", tag="w2sb", bufs=1)

        Es, Gs, prods = [], [], []
        for s in range(NES):
            sl = slice(s * ES, (s + 1) * ES)
            leak = leaks[s]
            l3 = leak.rearrange("p (b q) -> p b q", q=P)
            nc.vector.tensor_mul(
                l3, l3,
                fac[:, s * GRP:(s + 1) * GRP, None].to_broadcast([128, GRP, P]),
            )
            E = big.tile([128, ES], MM_RED, name="E", tag="E", bufs=6)
            Es.append(E)
            nc.scalar.activation(
                E, leak, AF.Exp, bias=bias_sb[:, pr:pr + 1], scale=LAM
            )
            # denominator rows for both N-halves, one psum pair-tile
            rp = rowp.tile([2, 2, 512], F32, name="rowps", tag="rowps")
            for h in range(2):
                nc.tensor.matmul(
                    rp[:, h, :NS], lhsT=onesM2, rhs=E[:, h * NS:(h + 1) * NS]
                )
            # T = denom/cap_len folded into the psum->sbuf copy (ACT)
            nc.scalar.activation(
                denom[:, sl].rearrange("r (h n) -> r h n", h=2),
                rp[:, :, :NS],
                AF.Copy,
                scale=invl_sb[:, pr:pr + 1],
            )
            # threshold broadcast + compare (both halves, one cmp)
            tb = colp.tile([128, 2, 512], F32, name="colps", tag="colps")
            for h, n in enumerate((2 * s, 2 * s + 1)):
                nc.tensor.matmul(
                    tb[:, h, :NS], lhsT=bcast2, rhs=denom[:, n * NS:(n + 1) * NS]
                )
            Gm = big.tile([128, ES], F32, name="Gm", tag="scr", bufs=8)
            nc.vector.tensor_tensor(
                Gm.rearrange("p (h n) -> p h n", h=2),
                E.rearrange("p (h n) -> p h n", h=2),
                tb[:, :, :NS],
                op=OP.is_gt,
            )
            G = big.tile([128, ES], MM_BIG, name="G", tag="G", bufs=8)
            Gs.append(G)
            nc.vector.tensor_mul(G, E, Gm)
            prod = big.tile([128, ES], MM_RED, name="prod", tag="leak", bufs=8)
            prods.append(prod)
            nc.gpsimd.tensor_mul(prod, G, raws[s])

        sqVs = []
        for s in range(NES):
            sl = slice(s * ES, (s + 1) * ES)
            # w12 rows
            rp = rowp.tile([2, 2, 512], F32, name="rowps", tag="rowps")
            for h in range(2):
                nc.tensor.matmul(
                    rp[:, h, :NS], lhsT=onesM2, rhs=prods[s][:, h * NS:(h + 1) * NS]
                )
            nc.scalar.copy(
                w12sb[:, sl].rearrange("r (h n) -> r h n", h=2), rp[:, :, :NS]
            )
            # V = C^T G, squared on ACT
            vp = colp.tile([128, 2, 512], F32, name="colps", tag="colps")
            for h in range(2):
                nc.tensor.matmul(
                    vp[:, h, :NS], lhsT=cch_sb, rhs=Gs[s][:, h * NS:(h + 1) * NS]
                )
            sqV = big.tile([128, ES], MM_RED, name="sqV", tag="scr", bufs=8)
            sqVs.append(sqV)
            nc.scalar.activation(
                sqV.rearrange("p (h n) -> p h n", h=2), vp[:, :, :NS], AF.Square
            )
        for s in range(NES):
            sl = slice(s * ES, (s + 1) * ES)
            rp = rowp.tile([2, 2, 512], F32, name="rowps", tag="rowps")
            for h in range(2):
                nc.tensor.matmul(
                    rp[:, h, :NS], lhsT=onesM2, rhs=sqVs[s][:, h * NS:(h + 1) * NS]
                )
            nc.vector.tensor_copy(
                w2sb[:, sl].rearrange("r (h n) -> r h n", h=2), rp[:, :, :NS]
            )

        nc.sync.dma_start(w12o.ap()[2 * pr:2 * pr + 2, :], w12sb)
        nc.sync.dma_start(w2o.ap()[2 * pr:2 * pr + 2, :], w2sb)

    # software-pipelined emission: pair pr+1's phase A ahead of pair pr's phase B
    emit_A(0)
    for pr in range(NPAIR):
        if pr + 1 < NPAIR:
            emit_A(pr + 1)
        emit_B(pr)


_ONESM2 = np.zeros((128, 2), np.float32)
_ONESM2[0:L, 0] = 1.0
_ONESM2[64:64 + L, 1] = 1.0
_BCAST2 = np.zeros((2, 128), np.float32)
_BCAST2[0, 0:64] = 1.0
_BCAST2[1, 64:128] = 1.0

_CACHE = {}


def _get_program():
    if "nc" not in _CACHE:
        _CACHE["nc"] = _build_device_program()
    return _CACHE["nc"]


def _host_inputs(im, s, cl):
    """Build per-core in_maps (host-side sharding + layout prep)."""
    imT = _round_for(MM_BIG, np.ascontiguousarray(im.reshape(B * P, D).T))
    in_maps = []
    for c in range(NCORES):
        s_loc = s[c * TLOC:(c + 1) * TLOC]
        cl_loc = cl[c * TLOC:(c + 1) * TLOC]
        sT = np.zeros((NPAIR, D, 128), np.float32)
        cc = np.zeros((NPAIR, 128, 128), np.float32)
        bm = np.full((128, NPAIR), MASK_BIAS, np.float32)
        iv = np.zeros((2, NPAIR), np.float32)
        for pr in range(NPAIR):
            for j in (0, 1):
                t = 2 * pr + j
                off = 64 * j
                st = s_loc[t]
                sT[pr, :, off:off + L] = st.T
                K = (st.astype(np.float64) @ st.astype(np.float64).T)
                C = np.linalg.cholesky(K).astype(np.float32)
                cc[pr, off:off + L, off:off + L] = C
                bm[off:off + L, pr] = np.where(
                    np.arange(L) < cl_loc[t], 0.0, MASK_BIAS
                ).astype(np.float32)
                iv[j, pr] = np.float32(1.0 / float(cl_loc[t]))
        in_maps.append({
            "imT": imT, "sT": _round_for(MM_BIG, sT), "cch": _round_for(MM_BIG, cc),
            "biasM": bm, "invl": iv, "onesM2": _ONESM2, "bcast2": _BCAST2,
        })
    return in_maps


def _host_tail(im, w12, w2sq):
    imf = im.reshape(B * P, D)
    w1 = np.sqrt(np.sum(imf * imf, axis=1, dtype=np.float32))
    w2 = np.sqrt(np.maximum(w2sq, 0.0))
    cos = w12 / np.maximum(w1[None, :] * w2, np.float32(EPS))
    cosr = cos.reshape(B, B, P)
    scores = np.sort(cosr, axis=-1)[..., P // 3:].mean(axis=-1, dtype=np.float32)
    d = np.diag(scores).copy()
    cs = np.maximum(np.float32(MARGIN) + scores - d[:, None], 0.0)
    ci = np.maximum(np.float32(MARGIN) + scores - d[None, :], 0.0)
    np.fill_diagonal(cs, 0.0)
    np.fill_diagonal(ci, 0.0)
    loss = cs.max(axis=1).sum(dtype=np.float32) + ci.max(axis=0).sum(dtype=np.float32)
    return np.asarray(loss, dtype=np.float32)


def kernel(im, s, cap_lens, _profile=False):
    im = np.ascontiguousarray(np.asarray(im, dtype=np.float32))
    s = np.ascontiguousarray(np.asarray(s, dtype=np.float32))
    cl = np.asarray(cap_lens).astype(np.int64)

    nc = _get_program()
    in_maps = _host_inputs(im, s, cl)
    kw = dict(trace=True) if _profile else {}
    res = bass_utils.run_bass_kernel_spmd(
        nc, in_maps, core_ids=list(range(NCORES)), **kw
    )
    w12 = np.concatenate([res.results[c]["w12o"] for c in range(NCORES)], axis=0)
    w2sq = np.concatenate([res.results[c]["w2o"] for c in range(NCORES)], axis=0)
    out = _host_tail(im, w12, w2sq)
    if _profile:
        return out, res
    return out
